# revision 1
# baseline (speedup 1.0000x reference)
"""RNN-T JointNetwork kernel for 8 Trainium2 NeuronCores.

reference:
    combined = f[:, :, None, :] + p[:, None, :, :]   # (B,T,U,H)
    h = relu(combined)
    logits = einsum('btuh,vh->btuv', h, W) + b        # (B,T,U,V)

Shapes: f (8,256,640) p (8,64,640) W (1024,640) b (1024,) -> out (8,256,64,1024) f32.

Sharding: data-parallel over B — core i computes batch i. W/b replicated.

Per-core program (SPMD, f32r matmuls):
  - inputs pre-transposed on host: ft=f[b].T (640,256), pt=p[b].T (640,64),
    wt=W.T (640,1024), bias replicated to (128,1024).
  - h_u[h,t] = relu(ft[h,t] + pt[h,u]) via ScalarE activation (bias = pt column).
  - logits[t, u, :] via PE: out[tile] = h_u[kchunk, tslice].T @ wt[kchunk, vslice]
    accumulated over 5 k-chunks into PSUM; DVE adds bias while copying PSUM->SBUF;
    staged SBUF tiles are DMA'd out 2 MiB at a time ((128 t) x (4 u) x (1024 v)).
"""

import numpy as np

import concourse.bass as bass
import concourse.mybir as mybir
import concourse.tile as tile
from concourse.bass_utils import run_bass_kernel_spmd
from concourse.vector_clock import ScopedClock

B, T, U, H, V = 8, 256, 64, 640, 1024
KC = H // 128          # 5 contraction chunks
TC = T // 128          # 2 t chunks
N_CORES = 8
UG = 4                 # u values staged per output DMA (2 MiB per DMA)
MM_DT = mybir.dt.float32r

_PATCHED = False


_MAX_WAITS = 1  # this walrus build rejects >1 sem-wait per instruction


def _spill_waits(nc, inst, add):
    """If `inst` carries more than _MAX_WAITS sem-waits, move the excess onto
    same-engine nops emitted (in program order) just before it."""
    si = inst.sync_info
    waits = list(si.on_wait) if si and si.on_wait else []
    if len(waits) <= _MAX_WAITS:
        return
    excess = waits[: len(waits) - _MAX_WAITS]
    inst.sync_info = mybir.SyncInfo(
        on_wait=waits[len(waits) - _MAX_WAITS :],
        on_update=list(si.on_update or []),
    )
    for i in range(0, len(excess), _MAX_WAITS):
        nop = mybir.InstNoOp(name=f"{inst.name}_spillw{i}", ins=[], outs=[])
        nop.engine = inst.engine
        nop.sync_info = mybir.SyncInfo(
            on_wait=excess[i : i + _MAX_WAITS], on_update=[]
        )
        nc.register_instruction(nop, overwrite=True)
        add(nop)


def _patch_tile_drain():
    """This walrus build's setupSyncWait rejects instructions carrying more
    than one sem-wait.  Tile freely emits several per instruction, so (a)
    split excess waits onto same-engine nops as instructions are committed
    into basic blocks, and (b) do the same for the end-of-kernel drain."""
    global _PATCHED
    if _PATCHED:
        return
    _PATCHED = True

    orig_add = tile.TileContext._add_instruction

    def _add_instruction(self, inst):
        _spill_waits(self.nc, inst, lambda n: orig_add(self, n))
        orig_add(self, inst)

    tile.TileContext._add_instruction = _add_instruction

    def _drain_and_barrier(self, tick_clock, wait_clock):
        nc = self.nc
        probe = nc.sync.nop(nofuse=True, hint="drain_wait_probe")
        wait_clock.add_sem_waits(
            probe.ins, ScopedClock({None: tick_clock.global_clock})
        )
        si = probe.ins.sync_info
        waits = list(si.on_wait) if si and si.on_wait else []
        if len(waits) > _MAX_WAITS:
            probe.ins.sync_info = mybir.SyncInfo(
                on_wait=waits[:_MAX_WAITS], on_update=list(si.on_update or [])
            )
            rest = waits[_MAX_WAITS:]
            for i in range(0, len(rest), _MAX_WAITS):
                extra = nc.sync.nop(nofuse=True, hint=f"drain_wait_{i}")
                extra.ins.sync_info = mybir.SyncInfo(
                    on_wait=rest[i : i + _MAX_WAITS], on_update=[]
                )
        nc.sync.drain()
        nc.all_engine_barrier()
        assert self.sems is not None
        popped = nc._tile_sem_poison_stack.pop()
        assert popped is self._sem_poison
        nc.clear_and_free_semaphores(list(self.sems.allocated().values()))
        nc.all_engine_barrier()

    tile.TileContext._drain_and_barrier = _drain_and_barrier


def build_program():
    """One SPMD NeuronCore program: (T,U,V) joint-network slice for one batch."""
    _patch_tile_drain()
    nc = bass.Bass()
    f32 = mybir.dt.float32

    ft = nc.dram_tensor("ft", [H, T], f32, kind="ExternalInput")
    pt = nc.dram_tensor("pt", [H, U], f32, kind="ExternalInput")
    wt = nc.dram_tensor("wt", [H, V], MM_DT, kind="ExternalInput")
    bias = nc.dram_tensor("bias", [128, V], f32, kind="ExternalInput")
    out = nc.dram_tensor("out", [T, U, V], f32, kind="ExternalOutput")

    ft_v = ft.rearrange("(k p) t -> p k t", p=128)
    pt_v = pt.rearrange("(k p) u -> p k u", p=128)
    wt_v = wt.rearrange("(k p) v -> p k v", p=128)

    with tile.TileContext(nc) as tc:
        with (
            tc.tile_pool(name="const", bufs=1) as cpool,
            tc.tile_pool(name="h", bufs=3) as hpool,
            tc.tile_pool(name="stage", bufs=3) as spool,
            tc.tile_pool(name="psum", bufs=8, space="PSUM") as ppool,
        ):
            ft_sb = cpool.tile([128, KC, T], f32)
            pt_sb = cpool.tile([128, KC, U], f32)
            wt_ks = [cpool.tile([128, V], MM_DT, name=f"wt_k{k}")
                     for k in range(KC)]
            bias_sb = cpool.tile([128, V], f32)
            nc.sync.dma_start(ft_sb[:], ft_v[:])
            nc.sync.dma_start(pt_sb[:], pt_v[:])
            for k in range(KC):
                nc.sync.dma_start(wt_ks[k][:], wt_v[:, k, :])
            nc.sync.dma_start(bias_sb[:], bias[:])


            for u0 in range(0, U, UG):
                stages = [spool.tile([128, UG, V], f32, tag=f"st{t_}",
                                     name=f"stage{t_}_{u0}")
                          for t_ in range(TC)]
                for j in range(UG):
                    u = u0 + j
                    h_u = hpool.tile([128, KC, T], MM_DT, tag="h")
                    for k in range(KC):
                        nc.scalar.activation(
                            h_u[:, k, :],
                            ft_sb[:, k, :],
                            mybir.ActivationFunctionType.Relu,
                            bias=pt_sb[:, k, u : u + 1],
                        )
                    for t_ in range(TC):
                        psums = [ppool.tile([128, 512], f32, tag="ps",
                                            name=f"ps{u}_{t_}_{h_}")
                                 for h_ in range(2)]
                        for k in range(KC):
                            lhsT = h_u[:, k, t_ * 128 : (t_ + 1) * 128]
                            for h_ in range(2):
                                nc.tensor.matmul(
                                    psums[h_][:],
                                    lhsT,
                                    wt_ks[k][:, h_ * 512 : (h_ + 1) * 512],
                                    start=(k == 0),
                                    stop=(k == KC - 1),
                                )
                        for h_ in range(2):
                            sl = slice(h_ * 512, (h_ + 1) * 512)
                            nc.vector.tensor_add(
                                stages[t_][:, j, sl],
                                psums[h_][:],
                                bias_sb[:, sl],
                            )
                for t_ in range(TC):
                    nc.sync.dma_start(
                        out[t_ * 128 : (t_ + 1) * 128, u0 : u0 + UG, :],
                        stages[t_][:],
                    )
    return nc


def kernel(f, p, W, b):
    f = np.asarray(f, np.float32)
    p = np.asarray(p, np.float32)
    W = np.asarray(W, np.float32)
    b = np.asarray(b, np.float32)

    nc = build_program()

    wt = np.ascontiguousarray(W.T)                      # (H, V)
    bias = np.ascontiguousarray(np.broadcast_to(b, (128, V)))
    in_maps = [
        {
            "ft": np.ascontiguousarray(f[i].T),         # (H, T)
            "pt": np.ascontiguousarray(p[i].T),         # (H, U)
            "wt": wt,
            "bias": bias,
        }
        for i in range(N_CORES)
    ]
    res = run_bass_kernel_spmd(nc, in_maps, list(range(N_CORES)))
    return np.stack([res.results[i]["out"] for i in range(N_CORES)], axis=0)



# revision 2
# speedup vs baseline: 1.0681x; 1.0681x over previous
"""RNN-T JointNetwork kernel for 8 Trainium2 NeuronCores.

reference:
    combined = f[:, :, None, :] + p[:, None, :, :]   # (B,T,U,H)
    h = relu(combined)
    logits = einsum('btuh,vh->btuv', h, W) + b        # (B,T,U,V)

Shapes: f (8,256,640) p (8,64,640) W (1024,640) b (1024,) -> out (8,256,64,1024) f32.

Sharding: data-parallel over B - core i computes batch i. W/b replicated.

Per-core program (SPMD, bf16 matmuls, fp32 PSUM accumulate):
  - host pre-permutes inputs so every DMA is contiguous:
      ft (128,5,256) f32 = f[b].T chunked, pt (128,5,64) f32, wt (5,128,1024)
      bf16 = W.T chunked, bias (128,1024) f32 replicated rows.
  - input DMAs issued from the ScalarE HWDGE ring (starts ~1.5us; the sync
    ring's preamble would delay them to ~8.6us).
  - ~64 warm-up matmuls on a memset tile keep the PE busy from ~1.5us so the
    HAM clock-gate reaches 8/8 before real matmuls start (saves the ~16us
    cold-throttle window) and real MMs issue back-to-back at warm rate.
  - h_u[h,t] = relu(ft[h,t] + pt[h,u]) via ScalarE activation, bf16 out
    (bf16 stationary operand => FWL weight loads, hidden by the PE reorder
    window behind the 512-col streams).
  - logits: per (u, t-chunk) accumulate 5 k-chunks x 2 v-halves into a
    2-bank PSUM tile; one DVE tensor_add drains 1024 cols + bias -> bf16
    stage tile; output DMA'd as bf16 (halves HBM write traffic), host
    upcasts to f32 (rounding ~0.4% of element magnitude, well inside the
    2e-2 gate).
"""

import numpy as np
import ml_dtypes

import concourse.bass as bass
import concourse.mybir as mybir
import concourse.tile as tile
from concourse.bass_utils import run_bass_kernel_spmd
from concourse.vector_clock import ScopedClock

B, T, U, H, V = 8, 256, 64, 640, 1024
KC = H // 128          # 5 contraction chunks
TC = T // 128          # 2 t chunks
N_CORES = 8
UG = 2                 # u values staged per output DMA (512 KiB bf16 per DMA)
WARM_MMS = 64          # N=128 warm-up matmuls to hold HAM at 8/8 during input DMA

_PATCHED = False


_MAX_WAITS = 1  # this walrus build rejects >1 sem-wait per instruction


def _spill_waits(nc, inst, add):
    """If `inst` carries more than _MAX_WAITS sem-waits, move the excess onto
    same-engine nops emitted (in program order) just before it."""
    si = inst.sync_info
    waits = list(si.on_wait) if si and si.on_wait else []
    if len(waits) <= _MAX_WAITS:
        return
    excess = waits[: len(waits) - _MAX_WAITS]
    inst.sync_info = mybir.SyncInfo(
        on_wait=waits[len(waits) - _MAX_WAITS :],
        on_update=list(si.on_update or []),
    )
    for i in range(0, len(excess), _MAX_WAITS):
        nop = mybir.InstNoOp(name=f"{inst.name}_spillw{i}", ins=[], outs=[])
        nop.engine = inst.engine
        nop.sync_info = mybir.SyncInfo(
            on_wait=excess[i : i + _MAX_WAITS], on_update=[]
        )
        nc.register_instruction(nop, overwrite=True)
        add(nop)


def _patch_tile_drain():
    """This walrus build's setupSyncWait rejects instructions carrying more
    than one sem-wait.  Tile freely emits several per instruction, so (a)
    split excess waits onto same-engine nops as instructions are committed
    into basic blocks, and (b) do the same for the end-of-kernel drain."""
    global _PATCHED
    if _PATCHED:
        return
    _PATCHED = True

    orig_add = tile.TileContext._add_instruction

    def _add_instruction(self, inst):
        _spill_waits(self.nc, inst, lambda n: orig_add(self, n))
        orig_add(self, inst)

    tile.TileContext._add_instruction = _add_instruction

    def _drain_and_barrier(self, tick_clock, wait_clock):
        nc = self.nc
        probe = nc.sync.nop(nofuse=True, hint="drain_wait_probe")
        wait_clock.add_sem_waits(
            probe.ins, ScopedClock({None: tick_clock.global_clock})
        )
        si = probe.ins.sync_info
        waits = list(si.on_wait) if si and si.on_wait else []
        if len(waits) > _MAX_WAITS:
            probe.ins.sync_info = mybir.SyncInfo(
                on_wait=waits[:_MAX_WAITS], on_update=list(si.on_update or [])
            )
            rest = waits[_MAX_WAITS:]
            for i in range(0, len(rest), _MAX_WAITS):
                extra = nc.sync.nop(nofuse=True, hint=f"drain_wait_{i}")
                extra.ins.sync_info = mybir.SyncInfo(
                    on_wait=rest[i : i + _MAX_WAITS], on_update=[]
                )
        nc.sync.drain()
        nc.all_engine_barrier()
        assert self.sems is not None
        popped = nc._tile_sem_poison_stack.pop()
        assert popped is self._sem_poison
        nc.clear_and_free_semaphores(list(self.sems.allocated().values()))
        nc.all_engine_barrier()

    tile.TileContext._drain_and_barrier = _drain_and_barrier


def build_program():
    """One SPMD NeuronCore program: (T,U,V) joint-network slice for one batch."""
    _patch_tile_drain()
    nc = bass.Bass()
    f32 = mybir.dt.float32
    bf16 = mybir.dt.bfloat16

    ft = nc.dram_tensor("ft", [128, KC, T], f32, kind="ExternalInput")
    pt = nc.dram_tensor("pt", [128, KC, U], f32, kind="ExternalInput")
    wt = nc.dram_tensor("wt", [KC, 128, V], bf16, kind="ExternalInput")
    bias = nc.dram_tensor("bias", [128, V], f32, kind="ExternalInput")
    out = nc.dram_tensor("out", [T, U, V], bf16, kind="ExternalOutput")

    with tile.TileContext(nc) as tc:
        with (
            tc.tile_pool(name="const", bufs=1) as cpool,
            tc.tile_pool(name="h", bufs=3) as hpool,
            tc.tile_pool(name="stage", bufs=3) as spool,
            tc.tile_pool(name="psum", bufs=4, space="PSUM") as ppool,
        ):
            # -- PE warm-up: memset a small bf16 tile, run back-to-back MMs so
            # the HAM un-throttles while the input DMAs are in flight.
            warm_sb = cpool.tile([128, 128], bf16, name="warm_sb")
            nc.gpsimd.memset(warm_sb[:], 0.0)
            warm_ps = ppool.tile([128, V], f32, tag="ps", name="warm_ps")
            for i in range(WARM_MMS):
                nc.tensor.matmul(
                    warm_ps[:, 0:128], warm_sb[:], warm_sb[:],
                    start=True, stop=True,
                )

            ft_sb = cpool.tile([128, KC, T], f32)
            pt_sb = cpool.tile([128, KC, U], f32)
            wt_ks = [cpool.tile([128, V], bf16, name=f"wt_k{k}")
                     for k in range(KC)]
            bias_sb = cpool.tile([128, V], f32)
            # Inputs on the ScalarE HWDGE ring, critical-path first.
            nc.scalar.dma_start(ft_sb[:], ft[:])
            nc.scalar.dma_start(pt_sb[:], pt[:])
            nc.scalar.dma_start(wt_ks[0][:], wt[0])
            nc.scalar.dma_start(wt_ks[1][:], wt[1])
            nc.scalar.dma_start(bias_sb[:], bias[:])
            for k in range(2, KC):
                nc.scalar.dma_start(wt_ks[k][:], wt[k])

            for u0 in range(0, U, UG):
                stages = [spool.tile([128, UG, V], bf16, tag=f"st{t_}",
                                     name=f"stage{t_}_{u0}")
                          for t_ in range(TC)]
                for j in range(UG):
                    u = u0 + j
                    h_u = hpool.tile([128, KC, T], bf16, tag="h")
                    for k in range(KC):
                        nc.scalar.activation(
                            h_u[:, k, :],
                            ft_sb[:, k, :],
                            mybir.ActivationFunctionType.Relu,
                            bias=pt_sb[:, k, u : u + 1],
                        )
                    for t_ in range(TC):
                        psum = ppool.tile([128, V], f32, tag="ps",
                                          name=f"ps{u}_{t_}")
                        for k in range(KC):
                            lhsT = h_u[:, k, t_ * 128 : (t_ + 1) * 128]
                            for h_ in range(2):
                                nc.tensor.matmul(
                                    psum[:, h_ * 512 : (h_ + 1) * 512],
                                    lhsT,
                                    wt_ks[k][:, h_ * 512 : (h_ + 1) * 512],
                                    start=(k == 0),
                                    stop=(k == KC - 1),
                                )
                        nc.vector.tensor_add(
                            stages[t_][:, j, :],
                            psum[:],
                            bias_sb[:],
                        )
                for t_ in range(TC):
                    nc.sync.dma_start(
                        out[t_ * 128 : (t_ + 1) * 128, u0 : u0 + UG, :],
                        stages[t_][:],
                    )
    return nc


def _prep_inputs(f, p, W, b):
    wt = np.ascontiguousarray(
        W.T.reshape(KC, 128, V).astype(ml_dtypes.bfloat16)
    )                                                   # (5,128,1024) bf16
    bias = np.ascontiguousarray(np.broadcast_to(b, (128, V)))
    in_maps = []
    for i in range(N_CORES):
        ft = np.ascontiguousarray(
            f[i].T.reshape(KC, 128, T).transpose(1, 0, 2)
        )                                               # (128,5,256) f32
        pt = np.ascontiguousarray(
            p[i].T.reshape(KC, 128, U).transpose(1, 0, 2)
        )                                               # (128,5,64) f32
        in_maps.append({"ft": ft, "pt": pt, "wt": wt, "bias": bias})
    return in_maps


def kernel(f, p, W, b):
    f = np.asarray(f, np.float32)
    p = np.asarray(p, np.float32)
    W = np.asarray(W, np.float32)
    b = np.asarray(b, np.float32)

    nc = build_program()
    in_maps = _prep_inputs(f, p, W, b)
    res = run_bass_kernel_spmd(nc, in_maps, list(range(N_CORES)))
    return np.stack(
        [res.results[i]["out"].astype(np.float32) for i in range(N_CORES)],
        axis=0,
    )
